# revision 1
# baseline (speedup 1.0000x reference)
"""Trainium2 Bass kernel for nn_ConvGen092: Conv2d(3->16,k2,VALID) + BatchNorm
(training stats) + ReLU + mean(tanh) pooled over [C,2,2] windows, flattened.

Sharding: data-parallel over batch, 8 images per core, 8 cores. One SPMD
program with two passes:
  Pass A: conv as block-diagonal matmul (8 conv rows x 16 ch -> 128 PSUM
          partitions per chunk); per-(g,c) sum(z) via DVE accum_out and
          sum(z^2) via ACT Square accum_out; fold to per-channel sums with a
          selector matmul; AllReduce 32 floats across the 8 cores.
  Finalize: A_c = gamma_c*rsqrt(var), B_c = beta_c - mean*A_c (conv bias
          cancels inside BN), broadcast to [128,1].
  Pass B: conv again (recompute is cheaper than spilling feat), ACT
          tanh(A*z+B), DVE relu, column-tiled selector matmul sums the 16
          channels (*1/64), DMA restack to row-major, DVE adds the (i,i+1)
          and (j,j+1) window shifts, DMA out.
Matmul operands bf16 (fp32 PSUM accumulate); stats and final sums fp32.
"""

from contextlib import ExitStack

import numpy as np

import concourse.bacc as bacc
import concourse.bass as bass  # noqa: F401
import concourse.mybir as mybir
import concourse.tile as tile

F32 = mybir.dt.float32
BF16 = mybir.dt.bfloat16
AX = mybir.AxisListType
OP = mybir.AluOpType
AF = mybir.ActivationFunctionType

BN_EPS = 1e-5
OC = 16   # out channels
CH = 3    # in channels
KK = 12   # ci*ki*kj contraction per group
W = 256   # image width
OW = W - 1  # conv output width (255)
G = 8     # conv rows per chunk
BPC = 4   # chunks per conv batch (z psum tile = 2 banks)


def _blocks(hp):
    """SBUF row-blocks covering padded rows [0, hp); each chunk's 9 rows must
    sit inside one block."""
    out = []
    r0 = 0
    while True:
        nr = min(128, hp - r0)
        out.append((r0, nr))
        last_i0 = r0 + nr - 9
        if r0 + nr >= hp:
            break
        r0 = ((last_i0 // G) + 1) * G
    return out


def build_nc(n_cores=8, imgs=8, h=256, exact_stats=True, compile_=True,
             reps=1, ablate=()):
    hp = h + 1            # one zero row of padding
    oh = h - 1            # conv output rows
    chunks = h // G       # last chunk computes one fake row
    nbatch = chunks // BPC
    assert chunks % BPC == 0 and nbatch >= 2
    assert nbatch % min(4, nbatch) == 0  # drain grouping
    blocks = _blocks(hp)
    nblk = len(blocks)
    c2b = []
    for c in range(chunks):
        i0 = c * G
        for bi, (r0, nr) in enumerate(blocks):
            if i0 >= r0 and i0 + G + 1 <= r0 + nr:
                c2b.append((bi, i0 - r0))
                break
    assert len(c2b) == chunks
    scols_img = (nbatch - 1) + 3
    scols = imgs * scols_img
    n_valid = imgs * oh * OW

    nc = bacc.Bacc("TRN2", target_bir_lowering=False, debug=False,
                   num_devices=n_cores)
    x_d = nc.dram_tensor("x_shard", [imgs, CH, hp, W], BF16, kind="ExternalInput")
    wblk_d = nc.dram_tensor("wblk_in", [12 * G, 128], BF16, kind="ExternalInput")
    ssel_d = nc.dram_tensor("ssel_in", [4, 128, 32], BF16, kind="ExternalInput")
    w2_d = nc.dram_tensor("w2_in", [2, 12 * G, OC], BF16, kind="ExternalInput")
    sela_d = nc.dram_tensor("sela_in", [128, OC], F32, kind="ExternalInput")
    g_d = nc.dram_tensor("gamma", [OC], F32, kind="ExternalInput")
    b_d = nc.dram_tensor("beta", [OC], F32, kind="ExternalInput")
    y_d = nc.dram_tensor("y_shard", [imgs, oh - 1, OW - 1], F32,
                         kind="ExternalOutput")

    with tile.TileContext(nc) as tc, ExitStack() as ctx:
        consts = ctx.enter_context(tc.tile_pool(name="consts", bufs=1))
        stats_p = ctx.enter_context(tc.tile_pool(name="stats", bufs=1))
        xpool = ctx.enter_context(tc.tile_pool(name="ximg", bufs=2))
        impool = ctx.enter_context(tc.tile_pool(name="im2col", bufs=imgs))
        work = ctx.enter_context(tc.tile_pool(name="work", bufs=2))
        spool = ctx.enter_context(tc.tile_pool(name="simg", bufs=2))
        zpool = ctx.enter_context(tc.tile_pool(name="zpsum", bufs=3, space="PSUM"))
        spsum = ctx.enter_context(tc.tile_pool(name="spsum", bufs=1, space="PSUM"))
        psum1 = ctx.enter_context(tc.tile_pool(name="psum1", bufs=1, space="PSUM"))
        dram = ctx.enter_context(tc.tile_pool(name="dram", bufs=1, space="DRAM"))

        # ---- static weights / selectors -----------------------------------
        wblk = consts.tile([12 * G, 128], BF16)
        ssel = [consts.tile([128, 32], BF16, tag=f"ssel{v}", name=f"ssel{v}")
                for v in range(4)]
        w2t = consts.tile([12 * G, OC], BF16)
        w2z = consts.tile([12 * G, OC], BF16)
        sela = consts.tile([128, OC], F32)
        avec = consts.tile([128, 1], F32)
        bvec = consts.tile([128, 1], F32)
        gam16 = consts.tile([OC, 1], F32)
        bet16 = consts.tile([OC, 1], F32)

        nc.gpsimd.dma_start(wblk[:], wblk_d.ap())
        for v in range(4):
            nc.gpsimd.dma_start(ssel[v][:], ssel_d.ap()[v])
        nc.gpsimd.dma_start(w2t[:], w2_d.ap()[0])
        nc.gpsimd.dma_start(w2z[:], w2_d.ap()[1])
        nc.gpsimd.dma_start(sela[:], sela_d.ap())
        nc.gpsimd.dma_start(gam16[:], g_d.ap().unsqueeze(1))
        nc.gpsimd.dma_start(bet16[:], b_d.ap().unsqueeze(1))

        stats_u = stats_p.tile([128, scols], F32)
        nc.vector.memset(stats_u[:], 0.0)

        for _rep in range(reps):
            # ---- helpers ------------------------------------------------------
            def load_im2col(img):
                # partition order: p = 32*ci + 16*ki + 8*kj + g (host permutes
                # wblk rows to match); src rows g are consecutive in DRAM.
                im = impool.tile([12 * G, chunks, OW], BF16, tag="im")
                im_r = im[:].rearrange("(ci ki kj g) ch j -> ci ki kj g ch j",
                                       ci=CH, ki=2, kj=2)
                xi = x_d.ap()[img]  # [CH, hp, W] bf16 in DRAM
                k = 0
                for ci in range(CH):
                    for ki in range(2):
                        for kj in range(2):
                            src = xi[ci, ki:ki + G * chunks, kj:kj + OW]
                            src = src.rearrange("(ch g) c -> g ch c", g=G)
                            eng = nc.sync if k % 2 == 0 else nc.scalar
                            eng.dma_start(im_r[ci, ki, kj], src)
                            k += 1
                return im[:].rearrange("p ch j -> p (ch j)")

            def conv_batch(im_flat, bt):
                z = zpool.tile([128, 1024], F32, tag="z")
                for q in range(2):
                    c0 = bt * BPC + 2 * q
                    nc.tensor.matmul(
                        z[:, 512 * q:512 * q + 2 * OW],
                        wblk[:],
                        im_flat[:, c0 * OW:c0 * OW + 2 * OW],
                        start=True, stop=True,
                    )
                return z

            def strided(ap):
                return ap.rearrange("p (q n) -> p q n", n=512)[:, :, 0:2 * OW]

            # ---- PASS A: stats ------------------------------------------------
            # sum(z) per channel via W2 selector matmuls accumulating in PSUM
            # (W2z zeroes the g=7 block to exclude the last chunk's fake row)
            zsum = psum1.tile([OC, 512], F32, tag="zsum")
            first_z = [True]

            def w2_acc(im_flat, w2sel, c0, ncols, last=False):
                nc.tensor.matmul(
                    zsum[:, 0:ncols], w2sel[:],
                    im_flat[:, c0 * OW:c0 * OW + ncols],
                    start=first_z[0], stop=last,
                )
                first_z[0] = False

            im_flats = {}
            for img in range(imgs):
                if 'passA' in ablate:
                    break
                im_flats[img] = im_flat = load_im2col(img)
                for bt in range(nbatch):
                    z = conv_batch(im_flat, bt)
                    sc0 = img * scols_img
                    if 'stats' in ablate:
                        continue
                    last_img = img == imgs - 1
                    if 'w2' in ablate:
                        if first_z[0]:
                            w2_acc(im_flat, w2t, 0, 2 * OW, last=True)
                        regions = ([] if 'sq' in ablate
                                   else [(lambda a: strided(a), 128, sc0)])
                    elif bt < nbatch - 1:
                        w2_acc(im_flat, w2t, bt * BPC, 2 * OW)
                        w2_acc(im_flat, w2t, bt * BPC + 2, 2 * OW)
                        regions = [(lambda a: strided(a), 128, sc0 + bt)]
                    else:
                        w2_acc(im_flat, w2t, bt * BPC, 2 * OW)
                        w2_acc(im_flat, w2t, bt * BPC + 2, OW)
                        w2_acc(im_flat, w2z, bt * BPC + 3, OW, last=last_img)
                        regions = [
                            (lambda a: a[:, 0:2 * OW], 128, sc0 + nbatch - 1),
                            (lambda a: a[:, 512:512 + OW], 128, sc0 + nbatch),
                            (lambda a: a[0:112, 512 + OW:512 + 2 * OW], 112,
                             sc0 + nbatch + 1),
                        ]
                    for slc, np_, col in regions:
                        scr_a = work.tile([128, 1024], BF16, tag="scr_a")
                        nc.scalar.activation(
                            slc(scr_a[:]), slc(z[:]), AF.Square,
                            accum_out=stats_u[0:np_, col:col + 1],
                        )

            # ---- finalize: stats -> A, B --------------------------------------
            zcol = stats_p.tile([128, 1], F32)
            nc.vector.tensor_reduce(zcol[:], stats_u[:], axis=AX.X, op=OP.add)
            # m16 borrows the unused last column of the zsum bank
            m16 = zsum[0:OC, 511:512]
            nc.tensor.matmul(m16, sela[:], zcol[:], start=True, stop=True,
                             skip_group_check=True)
            sums = stats_p.tile([OC, 2], F32)
            nc.vector.tensor_reduce(sums[:, 0:1], zsum[:, 0:2 * OW], axis=AX.X,
                                    op=OP.add)
            nc.vector.tensor_copy(sums[:, 1:2], m16)

            if exact_stats and n_cores > 1:
                cc_in = dram.tile([OC, 2], F32)
                cc_out = dram.tile([OC, 2], F32)
                nc.gpsimd.dma_start(cc_in[:], sums[:])
                nc.gpsimd.collective_compute(
                    "AllReduce", OP.add,
                    replica_groups=[list(range(n_cores))],
                    ins=[cc_in.opt()], outs=[cc_out.opt()],
                )
                nc.gpsimd.dma_start(sums[:], cc_out[:])
                inv_n = 1.0 / (n_cores * n_valid)
            else:
                inv_n = 1.0 / n_valid

            mz = stats_p.tile([OC, 1], F32)
            m2 = stats_p.tile([OC, 1], F32)
            var = stats_p.tile([OC, 1], F32)
            tmp = stats_p.tile([OC, 1], F32)
            rs = stats_p.tile([OC, 1], F32)
            a16 = stats_p.tile([OC, 1], F32)
            b16 = stats_p.tile([OC, 1], F32)
            nc.vector.tensor_scalar(mz[:], sums[:, 0:1], inv_n, None, OP.mult)
            nc.vector.tensor_scalar(m2[:], sums[:, 1:2], inv_n, None, OP.mult)
            nc.vector.tensor_mul(tmp[:], mz[:], mz[:])
            nc.vector.tensor_tensor(var[:], m2[:], tmp[:], OP.subtract)
            nc.vector.tensor_scalar(var[:], var[:], BN_EPS, None, OP.add)
            nc.scalar.activation(tmp[:], var[:], AF.Sqrt)
            nc.vector.reciprocal(rs[:], tmp[:])
            # one Newton polish: rs *= 1.5 - 0.5*var*rs^2
            nc.vector.tensor_mul(tmp[:], rs[:], rs[:])
            nc.vector.tensor_mul(tmp[:], tmp[:], var[:])
            nc.vector.tensor_scalar(tmp[:], tmp[:], -0.5, 1.5, OP.mult, OP.add)
            nc.vector.tensor_mul(rs[:], rs[:], tmp[:])
            nc.vector.tensor_mul(a16[:], rs[:], gam16[:])
            nc.vector.tensor_mul(tmp[:], mz[:], a16[:])
            nc.vector.tensor_tensor(b16[:], bet16[:], tmp[:], OP.subtract)
            for g in range(G):
                nc.gpsimd.dma_start(avec[OC * g:OC * g + OC, :], a16[:])
                nc.gpsimd.dma_start(bvec[OC * g:OC * g + OC, :], b16[:])

            # ---- PASS B: output -----------------------------------------------
            srows = oh + 1
            sblk = (srows + 127) // 128
            for img in range(imgs):
                if 'passB' in ablate:
                    break
                im_flat = im_flats.get(img) if im_flats else load_im2col(img)
                if im_flat is None:
                    im_flat = load_im2col(img)
                s_img = spool.tile([128, sblk * 256], F32, tag="simg")
                nc.vector.memset(s_img[:], 0.0)
                for bt in range(nbatch):
                    z = conv_batch(im_flat, bt)
                    if 'post' in ablate:
                        continue
                    t_sb = work.tile([128, 1024], BF16, tag="tanh")
                    nc.scalar.activation(strided(t_sb[:]), strided(z[:]), AF.Tanh,
                                         bias=bvec[:], scale=avec[:])
                    nc.vector.tensor_scalar(strided(t_sb[:]), strided(t_sb[:]),
                                            0.0, None, OP.max)
                    if 'chansum' in ablate:
                        continue
                    # chunk c = 4*bt+q lands identity-mapped at partitions
                    # 8*(c%16)+m of the half-image bank: col-group c%16//4,
                    # selector variant c%4
                    gb = min(4, nbatch)
                    if bt % gb == 0:
                        s_ps = spsum.tile([128, 512], F32, tag="sps")
                    Q = bt % gb
                    for q in range(BPC):
                        off = 512 * (q // 2) + OW * (q % 2)
                        nc.tensor.matmul(
                            s_ps[32 * Q:32 * Q + 32, 0:OW],
                            ssel[q][:], t_sb[:][:, off:off + OW],
                            start=(q == 0), stop=(q == BPC - 1),
                            tile_position=(0, 32 * Q),
                        )
                    if 'drain' in ablate:
                        continue
                    if bt % gb == gb - 1:
                        half = bt // gb
                        nc.vector.tensor_copy(
                            s_img[0:32 * gb, 256 * half:256 * half + OW],
                            s_ps[0:32 * gb, 0:OW],
                        )
                s_sh = spool.tile([128, sblk * 256], F32, tag="ssh")
                nc.vector.memset(s_sh[:], 0.0)
                for b in range(sblk):
                    cb = 256 * b
                    nrow = min(srows - 128 * b, 128)
                    if nrow > 1:
                        nc.sync.dma_start(s_sh[0:nrow - 1, cb:cb + 256],
                                          s_img[1:nrow, cb:cb + 256])
                    if 128 * (b + 1) < srows:
                        nc.sync.dma_start(s_sh[127:128, cb:cb + 256],
                                          s_img[0:1, cb + 256:cb + 512])
                r2 = spool.tile([128, sblk * 256], F32, tag="r2")
                nc.vector.tensor_add(r2[:], s_img[:], s_sh[:])
                o_sb = spool.tile([128, sblk * 256], F32, tag="osb")
                o_r = o_sb[:].rearrange("p (b n) -> p b n", n=256)
                r2_r = r2[:].rearrange("p (b n) -> p b n", n=256)
                nc.vector.tensor_add(o_r[:, :, 0:OW - 1], r2_r[:, :, 0:OW - 1],
                                     r2_r[:, :, 1:OW])
                orow = oh - 1
                for b in range(sblk):
                    nr = min(orow - 128 * b, 128)
                    if nr <= 0:
                        break
                    nc.sync.dma_start(
                        y_d.ap()[img, 128 * b:128 * b + nr, :],
                        o_sb[0:nr, 256 * b:256 * b + OW - 1],
                    )

    if compile_:
        nc.compile()
    return nc


def host_consts(conv_w):
    """Precompute blockdiag weights and selector matrices.

    Contraction row order: p = 32*ci + 16*ki + 8*kj + g."""
    import ml_dtypes
    w = np.asarray(conv_w, np.float32)  # [OC, CH, 2, 2]
    wblk = np.zeros((12 * G, 128), np.float32)
    for ci in range(CH):
        for ki in range(2):
            for kj in range(2):
                for g in range(G):
                    p = 32 * ci + 16 * ki + 8 * kj + g
                    wblk[p, OC * g:OC * g + OC] = w[:, ci, ki, kj]
    ssel = np.zeros((4, 128, 32), np.float32)
    for v in range(4):
        for m in range(G):
            ssel[v, OC * m:OC * m + OC, G * v + m] = 1.0 / 64.0
    sela = np.zeros((128, OC), np.float32)
    for g in range(G):
        for c in range(OC):
            sela[OC * g + c, c] = 1.0
    w2 = np.zeros((2, 12 * G, OC), np.float32)
    for ci in range(CH):
        for ki in range(2):
            for kj in range(2):
                for g in range(G):
                    p = 32 * ci + 16 * ki + 8 * kj + g
                    w2[0, p, :] = w[:, ci, ki, kj]
                    if g < 7:
                        w2[1, p, :] = w[:, ci, ki, kj]
    return (wblk.astype(ml_dtypes.bfloat16), ssel.astype(ml_dtypes.bfloat16),
            sela, w2.astype(ml_dtypes.bfloat16))


# ---------------------------------------------------------------------------
_CACHE = {}


def _get_nc():
    if "nc" not in _CACHE:
        _CACHE["nc"] = build_nc()
    return _CACHE["nc"]


def kernel(x, conv_w, conv_b, gamma, beta):
    import ml_dtypes
    from concourse.bass_utils import run_bass_kernel_spmd

    n_cores = 8
    x = np.asarray(x, dtype=np.float32)
    per = x.shape[0] // n_cores
    xp = np.zeros((x.shape[0], CH, W + 1, W), dtype=ml_dtypes.bfloat16)
    xp[:, :, :W, :] = x.astype(ml_dtypes.bfloat16)
    wblk, ssel, sela, w2 = host_consts(conv_w)
    in_maps = [
        {
            "x_shard": xp[c * per:(c + 1) * per],
            "wblk_in": wblk,
            "ssel_in": ssel,
            "sela_in": sela,
            "w2_in": w2,
            "gamma": np.asarray(gamma, np.float32),
            "beta": np.asarray(beta, np.float32),
        }
        for c in range(n_cores)
    ]
    nc = _get_nc()
    res = run_bass_kernel_spmd(nc, in_maps, list(range(n_cores)))
    out = np.concatenate([res.results[c]["y_shard"] for c in range(n_cores)],
                         axis=0)
    return out.reshape(-1)

